# revision 1
# baseline (speedup 1.0000x reference)
"""Trainium2 Bass kernel for nn_AttModule_mamba_cross (B=4,D=256,L=2048,E=512,N=16,K=7,R=16).

Sharding: 8 cores = 2 mamba blocks x 4 batches, one (block, batch) unit per core.
All cores run one SPMD program; a per-core flag selects whether the conv_ff(x)
path is blended in (self-block cores) or the raw input is used (cross-block
cores). Host sums each core pair's partial outputs.
"""
import numpy as np
import ml_dtypes

import concourse.bass as bass
import concourse.bacc as bacc
import concourse.mybir as mybir
import concourse.tile as tile
from concourse import masks
from concourse.bass_utils import run_bass_kernel_spmd

B, D, L = 4, 256, 2048
E, N, K = 512, 16, 7
R = 16
EPS = 1e-5

F32 = mybir.dt.float32
BF16 = mybir.dt.float16  # fp16: same 2x DVE rate as bf16, 8x better mantissa
MULT = mybir.AluOpType.mult
ADD = mybir.AluOpType.add
SUB = mybir.AluOpType.subtract
AF = mybir.ActivationFunctionType

DT = D // 128   # 2 d-tiles
ET = E // 128   # 4 e-tiles
TC = L // 512   # 4 t-chunks of 512


def build_nc():
    nc = bacc.Bacc("TRN2", target_bir_lowering=False, debug=False, num_devices=8)

    # ---- DRAM I/O ----
    xin_d = nc.dram_tensor("xin", [D, L], F32, kind="ExternalInput")
    maskb_d = nc.dram_tensor("maskb", [128, L], F32, kind="ExternalInput")
    flagv_d = nc.dram_tensor("flagv", [128, 1], F32, kind="ExternalInput")
    flag2_d = nc.dram_tensor("flag2", [128, 1], F32, kind="ExternalInput")
    ffw_d = nc.dram_tensor("ffw", [K, D, D], BF16, kind="ExternalInput")
    ffb_d = nc.dram_tensor("ffb", [D, 1], F32, kind="ExternalInput")
    w_inT_d = nc.dram_tensor("w_inT", [D, 2 * E], BF16, kind="ExternalInput")
    cw_d = nc.dram_tensor("cw", [E, K], F32, kind="ExternalInput")
    cb_d = nc.dram_tensor("cb", [E, 1], F32, kind="ExternalInput")
    w_xT_d = nc.dram_tensor("w_xT", [E, R + 2 * N], BF16, kind="ExternalInput")
    w_dtT_d = nc.dram_tensor("w_dtT", [R, E], F32, kind="ExternalInput")
    dtb_d = nc.dram_tensor("dtb", [E, 1], F32, kind="ExternalInput")
    aneg_d = nc.dram_tensor("aneg", [E, N], F32, kind="ExternalInput")
    dp_d = nc.dram_tensor("dp", [E, 1], F32, kind="ExternalInput")
    w_outT_d = nc.dram_tensor("w_outT", [E, D], BF16, kind="ExternalInput")
    out_d = nc.dram_tensor("out", [D, L], F32, kind="ExternalOutput")

    with tile.TileContext(nc) as tc:
        _emit(nc, tc, locals())
    nc.compile()
    return nc


def _emit(nc, tc, d):
    xin_d, maskb_d, flagv_d, flag2_d = d["xin_d"], d["maskb_d"], d["flagv_d"], d["flag2_d"]
    ffw_d, ffb_d, w_inT_d = d["ffw_d"], d["ffb_d"], d["w_inT_d"]
    cw_d, cb_d, w_xT_d, w_dtT_d = d["cw_d"], d["cb_d"], d["w_xT_d"], d["w_dtT_d"]
    dtb_d, aneg_d, dp_d, w_outT_d, out_d = d["dtb_d"], d["aneg_d"], d["dp_d"], d["w_outT_d"], d["out_d"]

    _pools = []

    def pool(name, bufs, space="SBUF"):
        p = tc.alloc_tile_pool(name=name, bufs=bufs, space=space)
        _pools.append(p)
        return p

    wpool = pool("weights", 1)          # persistent small weights/constants
    big = pool("big", 1)                # persistent big activations
    chunk = pool("chunk512", 2)         # transient [128,512] tiles
    psmall = pool("psum_mm", 2, space="PSUM")    # [128,512] matmul tiles
    mmwp = pool("psum_mmw", 2, space="PSUM")     # [128,1024] prelude matmul tiles
    dramp = pool("dram", 1, space="DRAM")
    # stage pools, created in reverse order of release (stack allocator is LIFO)
    stageA = pool("stageA", 1)          # bcsrc (released after x_proj DMA)
    stage2 = pool("stage2", 1)          # xp halo + diag_cw (released after dwconv)
    stage1 = pool("stage1", 1)          # xin padded (released after norm)
    ntmp = pool("ntmp", 2)              # norm temp tiles (released after norm)
    wff = pool("wff", 1)                # conv_ff weights (released after conv_ff)

    # ---------------- persistent weights ----------------
    maskb = big.tile([128, L], F32, tag="maskb", name="maskb")
    nc.sync.dma_start(maskb[:], maskb_d[:])
    flagv = wpool.tile([128, 1], F32, tag="flagv", name="flagv")
    flag2 = wpool.tile([128, 1], F32, tag="flag2", name="flag2")
    nc.sync.dma_start(flagv[:], flagv_d[:])
    nc.sync.dma_start(flag2[:], flag2_d[:])
    ffb = [wpool.tile([128, 1], F32, tag=f"ffb{i}", name=f"ffb{i}") for i in range(DT)]
    for i in range(DT):
        nc.sync.dma_start(ffb[i][:], ffb_d[i * 128:(i + 1) * 128, :])
    w_inT = [wpool.tile([128, 2 * E], BF16, tag=f"w_inT{i}", name=f"w_inT{i}") for i in range(DT)]
    for i in range(DT):
        nc.sync.dma_start(w_inT[i][:], w_inT_d[i * 128:(i + 1) * 128, :])
    cw = [wpool.tile([128, K], F32, tag=f"cw{i}", name=f"cw{i}") for i in range(ET)]
    cb = [wpool.tile([128, 1], F32, tag=f"cb{i}", name=f"cb{i}") for i in range(ET)]
    dtb = [wpool.tile([128, 1], F32, tag=f"dtb{i}", name=f"dtb{i}") for i in range(ET)]
    dp = [wpool.tile([128, 1], F32, tag=f"dp{i}", name=f"dp{i}") for i in range(ET)]
    aneg = [wpool.tile([128, N], F32, tag=f"aneg{i}", name=f"aneg{i}") for i in range(ET)]
    w_xT = [wpool.tile([128, R + 2 * N], BF16, tag=f"w_xT{i}", name=f"w_xT{i}") for i in range(ET)]
    w_outT = [wpool.tile([128, D], BF16, tag=f"w_outT{i}", name=f"w_outT{i}") for i in range(ET)]
    for i in range(ET):
        sl = slice(i * 128, (i + 1) * 128)
        nc.sync.dma_start(cw[i][:], cw_d[sl, :])
        nc.sync.dma_start(cb[i][:], cb_d[sl, :])
        nc.sync.dma_start(dtb[i][:], dtb_d[sl, :])
        nc.sync.dma_start(dp[i][:], dp_d[sl, :])
        nc.sync.dma_start(aneg[i][:], aneg_d[sl, :])
        nc.sync.dma_start(w_xT[i][:], w_xT_d[sl, :])
        nc.sync.dma_start(w_outT[i][:], w_outT_d[sl, :])
    w_dtT = wpool.tile([R, E], F32, tag="w_dtT", name="w_dtT")
    nc.sync.dma_start(w_dtT[:], w_dtT_d[:])

    ident = wpool.tile([128, 128], F32, tag="ident", name="ident")
    masks.make_identity(nc, ident[:])
    identb = wpool.tile([128, 128], BF16, tag="identb", name="identb")
    nc.vector.tensor_copy(identb[:], ident[:])
    diag_dp = [wpool.tile([128, 128], BF16, tag=f"ddp{e}", name=f"ddp{e}") for e in range(ET)]
    for e in range(ET):
        nc.vector.tensor_scalar_mul(diag_dp[e][:], ident[:], dp[e][:])
    diag_cw = [[stage2.tile([128, 128], BF16, tag=f"dcw{e}_{k}", name=f"dcw{e}_{k}") for k in range(K)]
               for e in range(ET)]
    for e in range(ET):
        for k in range(K):
            nc.vector.tensor_scalar_mul(diag_cw[e][k][:], ident[:], cw[e][:, k:k + 1])

    # ---------------- stage1: input + conv_ff weights ----------------
    xin = [stage1.tile([128, L + 6], F32, tag=f"xinpad{i}", name=f"xinpad{i}") for i in range(DT)]
    for i in range(DT):
        nc.gpsimd.memset(xin[i][:, 0:3], 0.0)
        nc.gpsimd.memset(xin[i][:, L + 3:], 0.0)
        nc.sync.dma_start(xin[i][:, 3:L + 3], xin_d[i * 128:(i + 1) * 128, :])
    ffw = [wff.tile([128, K * D], BF16, tag=f"ffw{i}", name=f"ffw{i}") for i in range(DT)]
    for i in range(DT):
        nc.sync.dma_start(
            ffw[i][:].rearrange("p (k d) -> p k d", k=K),
            ffw_d[:, i * 128:(i + 1) * 128, :].rearrange("k p d -> p k d"),
        )
    xin16 = [wff.tile([128, L + 6], BF16, tag=f"xin16_{i}", name=f"xin16_{i}") for i in range(DT)]
    for i in range(DT):
        nc.vector.tensor_copy(xin16[i][:], xin[i][:])

    # ---------------- conv_ff + residual ----------------
    resid = [big.tile([128, L], F32, tag=f"resid{i}", name=f"resid{i}") for i in range(DT)]
    for do in range(DT):
        for t in range(TC):
            ps = mmwp.tile([128, 512], F32, tag="mmw", name="psmmw")
            nmm = K * DT
            i = 0
            for k in range(K):
                for di in range(DT):
                    nc.tensor.matmul(
                        ps[:],
                        ffw[di][:, k * D + do * 128: k * D + (do + 1) * 128],
                        xin16[di][:, t * 512 + k: t * 512 + k + 512],
                        start=(i == 0), stop=(i == nmm - 1),
                    )
                    i += 1
            cchunk = chunk.tile([128, 512], F32, tag="convrelu", name="convrelu")
            nc.scalar.activation(cchunk[:], ps[:], AF.Relu, bias=ffb[do][:])
            nc.vector.scalar_tensor_tensor(
                resid[do][:, t * 512:(t + 1) * 512], cchunk[:], 1.0,
                xin[do][:, 3 + t * 512: 3 + (t + 1) * 512], MULT, ADD,
            )
    wff.release()

    # ---------------- instance norm + mask -> mamba_in (bf16) ----------------
    mamba_in = [big.tile([128, L], BF16, tag=f"mambain{i}", name=f"mambain{i}") for i in range(DT)]
    for i in range(DT):
        nin = ntmp.tile([128, L], F32, tag="t2048", name="nin")
        nc.vector.tensor_scalar_mul(nin[:], xin[i][:, 3:L + 3], flag2[:])
        nc.vector.scalar_tensor_tensor(nin[:], resid[i][:], flagv[:], nin[:], MULT, ADD)
        ssum = wpool.tile([128, 1], F32, tag=f"ssum{i}", name=f"ssum{i}")
        ssq = wpool.tile([128, 1], F32, tag=f"ssq{i}", name=f"ssq{i}")
        trash = ntmp.tile([128, L], F32, tag="t2048", name="trash")
        nc.vector.tensor_reduce(ssum[:], nin[:], mybir.AxisListType.X, ADD)
        nc.scalar.activation(trash[:], nin[:], AF.Square, accum_out=ssq[:])
        mean = wpool.tile([128, 1], F32, tag=f"mean{i}", name=f"mean{i}")
        var = wpool.tile([128, 1], F32, tag=f"var{i}", name=f"var{i}")
        nc.vector.tensor_scalar_mul(mean[:], ssum[:], 1.0 / L)
        nc.vector.tensor_scalar_mul(var[:], ssq[:], 1.0 / L)
        msq = wpool.tile([128, 1], F32, tag=f"msq{i}", name=f"msq{i}")
        nc.vector.tensor_tensor(msq[:], mean[:], mean[:], MULT)
        nc.vector.scalar_tensor_tensor(var[:], msq[:], -1.0, var[:], MULT, ADD)
        nc.vector.tensor_scalar_add(var[:], var[:], EPS)
        inv = wpool.tile([128, 1], F32, tag=f"inv{i}", name=f"inv{i}")
        nc.vector.reciprocal(inv[:], var[:])
        nc.scalar.sqrt(inv[:], inv[:])
        muinv = wpool.tile([128, 1], F32, tag=f"muinv{i}", name=f"muinv{i}")
        nc.vector.tensor_tensor(muinv[:], mean[:], inv[:], MULT)
        nc.vector.tensor_scalar(nin[:], nin[:], inv[:], muinv[:], MULT, SUB)
        nc.vector.tensor_tensor(mamba_in[i][:], nin[:], maskb[:], MULT)
    ntmp.release()
    stage1.release()

    # ---------------- in_proj (xp half) ----------------
    xp = [stage2.tile([128, L + 6], BF16, tag=f"xp{e}", name=f"xp{e}") for e in range(ET)]
    for e in range(ET):
        nc.gpsimd.memset(xp[e][:, 0:6], 0.0)
        for t in range(TC):
            ps = mmwp.tile([128, 512], F32, tag="mmw", name="psmmw")
            for di in range(DT):
                nc.tensor.matmul(
                    ps[:], w_inT[di][:, e * 128:(e + 1) * 128],
                    mamba_in[di][:, t * 512:(t + 1) * 512],
                    start=(di == 0), stop=(di == DT - 1),
                )
            nc.scalar.copy(xp[e][:, 6 + t * 512: 6 + (t + 1) * 512], ps[:])

    # ---------------- depthwise causal conv + silu -> u ----------------
    u = [big.tile([128, L], BF16, tag=f"u{e}", name=f"u{e}") for e in range(ET)]
    for e in range(ET):
        for t in range(TC):
            ps = mmwp.tile([128, 512], F32, tag="mmw", name="psmmw")
            for k in range(K):
                nc.tensor.matmul(
                    ps[:], diag_cw[e][k][:],
                    xp[e][:, t * 512 + k: t * 512 + k + 512],
                    start=(k == 0), stop=(k == K - 1),
                )
            vv = chunk.tile([128, 512], BF16, tag="vv", name="vv")
            nc.scalar.activation(vv[:], ps[:], AF.Identity, bias=cb[e][:])
            sg = chunk.tile([128, 512], BF16, tag="sg", name="sg")
            nc.scalar.activation(sg[:], ps[:], AF.Sigmoid, bias=cb[e][:])
            nc.vector.tensor_tensor(u[e][:, t * 512:(t + 1) * 512], vv[:], sg[:], MULT)
    stage2.release()

    # ---------------- x_proj -> xdblR + B/C rows to DRAM ----------------
    xdblR = big.tile([R, L], F32, tag="xdblR", name="xdblR")
    bcsrc = stageA.tile([2 * N, L], BF16, tag="bcsrc", name="bcsrc")
    for t in range(TC):
        ps = mmwp.tile([R + 2 * N, 512], F32, tag="mmw", name="psmmx")
        for e in range(ET):
            nc.tensor.matmul(
                ps[:], w_xT[e][:], u[e][:, t * 512:(t + 1) * 512],
                start=(e == 0), stop=(e == ET - 1),
            )
        nc.vector.tensor_copy(bcsrc[:, t * 512:(t + 1) * 512], ps[0:2 * N, :])
        nc.vector.tensor_copy(xdblR[:, t * 512:(t + 1) * 512], ps[2 * N:2 * N + R, :])
    bc_dram = dramp.tile([2 * N, L], BF16, tag="bcdram", name="bcdram")
    nc.sync.dma_start(bc_dram[:], bcsrc[:])

    # ---------------- dt_proj + softplus -> dt ----------------
    dt = [big.tile([128, L], BF16, tag=f"dt{e}", name=f"dt{e}") for e in range(ET)]
    for e in range(ET):
        for t in range(TC):
            ps = psmall.tile([128, 512], F32, tag="mm", name="psmm")
            nc.tensor.matmul(
                ps[:], w_dtT[:, e * 128:(e + 1) * 128],
                xdblR[:, t * 512:(t + 1) * 512], start=True, stop=True,
            )
            lntmp = chunk.tile([128, 512], F32, tag="lntmp", name="lntmp")
            nc.scalar.activation(lntmp[:], ps[:], AF.Sigmoid, bias=dtb[e][:], scale=-1.0)
            nc.scalar.activation(dt[e][:, t * 512:(t + 1) * 512], lntmp[:], AF.Ln)

    # ---------------- w = dt * u ----------------
    w = [big.tile([128, L], BF16, tag=f"w{e}", name=f"w{e}") for e in range(ET)]
    for e in range(ET):
        nc.vector.tensor_tensor(w[e][:], dt[e][:], u[e][:], MULT)
    stageA.release()

    # ---------------- selective scan ----------------
    mmwp.release()
    pbig = pool("psum_y", 1, space="PSUM")       # [128,2048] y accumulator
    scanp = pool("scan", 3)
    bcp = pool("bcast", 2)
    yg = []
    for e in range(ET):
        py = pbig.tile([128, L], F32, tag="py", name=f"py{e}")
        for t in range(TC):
            nc.tensor.matmul(
                py[:, t * 512:(t + 1) * 512], diag_dp[e][:],
                u[e][:, t * 512:(t + 1) * 512], start=True, stop=False,
            )
        for n in range(N):
            b_bc = bcp.tile([128, L], BF16, tag="bbc", name="bbc")
            c_bc = bcp.tile([128, L], BF16, tag="cbc", name="cbc")
            nc.sync.dma_start(b_bc[:], bc_dram[n, :].partition_broadcast(128))
            nc.sync.dma_start(c_bc[:], bc_dram[N + n, :].partition_broadcast(128))
            dA = scanp.tile([128, L], F32, tag="dA", name="dA")
            nc.scalar.activation(dA[:], dt[e][:], AF.Exp, scale=aneg[e][:, n:n + 1])
            z = scanp.tile([128, L], BF16, tag="z", name="z")
            nc.vector.tensor_tensor(z[:], w[e][:], b_bc[:], MULT)
            h = scanp.tile([128, L], BF16, tag="h", name="h")
            nc.vector.tensor_tensor_scan(h[:], dA[:], z[:], 0.0, MULT, ADD)
            yp = scanp.tile([128, L], BF16, tag="yp", name="yp")
            nc.vector.tensor_tensor(yp[:], h[:], c_bc[:], MULT)
            for t in range(TC):
                nc.tensor.matmul(
                    py[:, t * 512:(t + 1) * 512], identb[:],
                    yp[:, t * 512:(t + 1) * 512],
                    start=False, stop=(n == N - 1),
                )
        # ---- gating: yg = py * silu(z_gate) ----
        yge = big.tile([128, L], BF16, tag=f"u{e}", name=f"yg{e}")
        yg.append(yge)
        for t in range(TC):
            ps = psmall.tile([128, 512], F32, tag="mm", name="psmm")
            for di in range(DT):
                nc.tensor.matmul(
                    ps[:], w_inT[di][:, E + e * 128: E + (e + 1) * 128],
                    mamba_in[di][:, t * 512:(t + 1) * 512],
                    start=(di == 0), stop=(di == DT - 1),
                )
            sgz = chunk.tile([128, 512], BF16, tag="sgz", name="sgz")
            nc.scalar.activation(sgz[:], ps[:], AF.Sigmoid)
            zs = chunk.tile([128, 512], BF16, tag="zs", name="zs")
            nc.vector.tensor_tensor(zs[:], sgz[:], ps[:], MULT)
            nc.vector.tensor_tensor(
                yge[:, t * 512:(t + 1) * 512],
                py[:, t * 512:(t + 1) * 512], zs[:], MULT,
            )

    # ---------------- out_proj + combine + store ----------------
    for do in range(DT):
        for t in range(TC):
            ps = psmall.tile([128, 512], F32, tag="mm", name="psmm")
            for e in range(ET):
                nc.tensor.matmul(
                    ps[:], w_outT[e][:, do * 128:(do + 1) * 128],
                    yg[e][:, t * 512:(t + 1) * 512],
                    start=(e == 0), stop=(e == ET - 1),
                )
            o1 = chunk.tile([128, 512], F32, tag="o1", name="o1")
            nc.vector.scalar_tensor_tensor(
                o1[:], resid[do][:, t * 512:(t + 1) * 512], flagv[:], ps[:], MULT, ADD,
            )
            ofin = chunk.tile([128, 512], F32, tag="ofin", name="ofin")
            nc.vector.tensor_tensor(ofin[:], o1[:], maskb[:, t * 512:(t + 1) * 512], MULT)
            nc.sync.dma_start(out_d[do * 128:(do + 1) * 128, t * 512:(t + 1) * 512], ofin[:])

    for p in reversed(_pools):
        if not p._released:
            p.release()


_NC_CACHE = {}


def _get_nc():
    if "nc" not in _NC_CACHE:
        _NC_CACHE["nc"] = build_nc()
    return _NC_CACHE["nc"]


def _core_inputs(blk, b, inputs):
    pfx = "s_" if blk == 0 else "c_"
    xin = inputs["x"][b] if blk == 0 else inputs["encoder_states"][b]
    f = 1.0 if blk == 0 else 0.0
    g = lambda k: np.asarray(inputs[pfx + k])
    aneg = np.exp(g("A_log"))  # = -A; dA=exp(A*dt)=exp((-A)*lnsig)
    return {
        "xin": np.ascontiguousarray(xin, np.float32),
        "maskb": np.ascontiguousarray(
            np.broadcast_to(inputs["padding_mask"][b][None, :], (128, L)), np.float32),
        "flagv": np.full((128, 1), f, np.float32),
        "flag2": np.full((128, 1), 1.0 - 2.0 * f, np.float32),
        "ffw": np.ascontiguousarray(np.asarray(inputs["ff_w"]).transpose(2, 1, 0)).astype(np.float16),
        "ffb": np.asarray(inputs["ff_b"]).reshape(D, 1).astype(np.float32),
        "w_inT": np.ascontiguousarray(g("in_proj_w").T).astype(np.float16),
        "cw": np.ascontiguousarray(g("conv_w").reshape(E, K), np.float32),
        "cb": g("conv_b").reshape(E, 1).astype(np.float32),
        "w_xT": np.ascontiguousarray(
            g("x_proj_w").T[:, list(range(R, R + 2 * N)) + list(range(R))]
        ).astype(np.float16),
        "w_dtT": np.ascontiguousarray(g("dt_proj_w").T, np.float32),
        "dtb": (-g("dt_proj_b")).reshape(E, 1).astype(np.float32),
        "aneg": np.ascontiguousarray(aneg, np.float32),
        "dp": (-g("D")).reshape(E, 1).astype(np.float32),
        "w_outT": np.ascontiguousarray(-g("out_proj_w").T).astype(np.float16),
    }


def kernel(**inputs):
    nc = _get_nc()
    in_maps = []
    for b in range(B):
        in_maps.append(_core_inputs(0, b, inputs))  # core 2b: self block
        in_maps.append(_core_inputs(1, b, inputs))  # core 2b+1: cross block
    res = run_bass_kernel_spmd(nc, in_maps, core_ids=list(range(8)))
    out = np.empty((B, D, L), np.float32)
    for b in range(B):
        out[b] = res.results[2 * b]["out"] + res.results[2 * b + 1]["out"]
    return out



# revision 2
# speedup vs baseline: 1.0006x; 1.0006x over previous
"""Trainium2 Bass kernel for nn_AttModule_mamba_cross (B=4,D=256,L=2048,E=512,N=16,K=7,R=16).

Sharding: 8 cores = 2 mamba blocks x 4 batches, one (block, batch) unit per core.
All cores run one SPMD program; a per-core flag selects whether the conv_ff(x)
path is blended in (self-block cores) or the raw input is used (cross-block
cores). Host sums each core pair's partial outputs.

V-engine-optimized rewrite: the DVE is the bottleneck (64 scans + z/yp muls),
so everything else is pushed to Scalar (Silu/Softplus/Exp activations) and
Tensor (identity-matmul accumulation), fp16 throughout, and the gate/silu
work is hoisted out of the scan loop so Scalar runs Exp-only there.
padding_mask is all-ones per the problem spec (fill: ones); it is applied
only at the final output store.
"""
import numpy as np

import concourse.bass as bass
import concourse.bacc as bacc
import concourse.mybir as mybir
import concourse.tile as tile
from concourse import masks
from concourse.bass_utils import run_bass_kernel_spmd

B, D, L = 4, 256, 2048
E, N, K = 512, 16, 7
R = 16
EPS = 1e-5

F32 = mybir.dt.float32
FP16 = mybir.dt.float16
MULT = mybir.AluOpType.mult
ADD = mybir.AluOpType.add
SUB = mybir.AluOpType.subtract
AF = mybir.ActivationFunctionType

DT = D // 128   # 2 d-tiles
ET = E // 128   # 4 e-tiles
TC = L // 512   # 4 t-chunks of 512


def build_nc():
    nc = bacc.Bacc("TRN2", target_bir_lowering=False, debug=False, num_devices=8)

    # ---- DRAM I/O ----
    xin_d = nc.dram_tensor("xin", [D, L], F32, kind="ExternalInput")
    maskb_d = nc.dram_tensor("maskb", [128, L], FP16, kind="ExternalInput")
    flagv_d = nc.dram_tensor("flagv", [128, 1], F32, kind="ExternalInput")
    flag2_d = nc.dram_tensor("flag2", [128, 1], F32, kind="ExternalInput")
    ffw_d = nc.dram_tensor("ffw", [K, D, D], FP16, kind="ExternalInput")
    ffb_d = nc.dram_tensor("ffb", [D, 1], F32, kind="ExternalInput")
    w_inT_d = nc.dram_tensor("w_inT", [D, 2 * E], FP16, kind="ExternalInput")
    cw_d = nc.dram_tensor("cw", [E, K], F32, kind="ExternalInput")
    cb_d = nc.dram_tensor("cb", [E, 1], F32, kind="ExternalInput")
    w_xT_d = nc.dram_tensor("w_xT", [E, R + 2 * N], FP16, kind="ExternalInput")
    w_dtT_d = nc.dram_tensor("w_dtT", [R, E], FP16, kind="ExternalInput")
    dtb_d = nc.dram_tensor("dtb", [E, 1], F32, kind="ExternalInput")
    aneg_d = nc.dram_tensor("aneg", [E, N], F32, kind="ExternalInput")
    dp_d = nc.dram_tensor("dp", [E, 1], F32, kind="ExternalInput")
    w_outT_d = nc.dram_tensor("w_outT", [E, D], FP16, kind="ExternalInput")
    out_d = nc.dram_tensor("out", [D, L], F32, kind="ExternalOutput")

    with tile.TileContext(nc) as tc:
        _emit(nc, tc, locals())
    nc.compile()
    return nc


def _emit(nc, tc, d):
    xin_d, maskb_d, flagv_d, flag2_d = d["xin_d"], d["maskb_d"], d["flagv_d"], d["flag2_d"]
    ffw_d, ffb_d, w_inT_d = d["ffw_d"], d["ffb_d"], d["w_inT_d"]
    cw_d, cb_d, w_xT_d, w_dtT_d = d["cw_d"], d["cb_d"], d["w_xT_d"], d["w_dtT_d"]
    dtb_d, aneg_d, dp_d, w_outT_d, out_d = d["dtb_d"], d["aneg_d"], d["dp_d"], d["w_outT_d"], d["out_d"]

    _pools = []

    def pool(name, bufs, space="SBUF"):
        p = tc.alloc_tile_pool(name=name, bufs=bufs, space=space)
        _pools.append(p)
        return p

    wpool = pool("weights", 1)          # persistent small weights/constants
    big = pool("big", 1)                # persistent big activations
    chunk = pool("chunk512", 2)         # transient [128,512] tiles
    psmall = pool("psum_mm", 2, space="PSUM")    # [128,512] matmul tiles
    mmwp = pool("psum_mmw", 2, space="PSUM")     # [128,512] prelude matmul tiles
    dramp = pool("dram", 1, space="DRAM")
    # stage pools, created in reverse order of release (stack allocator is LIFO)
    stage2 = pool("stage2", 1)          # xp halo + diag_cw (released after dwconv)
    stage3 = pool("stage3", 1)          # mamba_in (released after gate)
    wff = pool("wff", 1)                # conv_ff weights + xin16 + convout (released after norm)
    stage1 = pool("stage1", 1)          # xin f32 (released after cast)

    # ---------------- persistent weights ----------------
    maskb = wpool.tile([128, L], FP16, tag="maskb", name="maskb")
    nc.sync.dma_start(maskb[:], maskb_d[:])
    flagv = wpool.tile([128, 1], F32, tag="flagv", name="flagv")
    flag2 = wpool.tile([128, 1], F32, tag="flag2", name="flag2")
    nc.sync.dma_start(flagv[:], flagv_d[:])
    nc.sync.dma_start(flag2[:], flag2_d[:])
    ffb = [wpool.tile([128, 1], F32, tag=f"ffb{i}", name=f"ffb{i}") for i in range(DT)]
    for i in range(DT):
        nc.sync.dma_start(ffb[i][:], ffb_d[i * 128:(i + 1) * 128, :])
    w_inT = [wpool.tile([128, 2 * E], FP16, tag=f"w_inT{i}", name=f"w_inT{i}") for i in range(DT)]
    for i in range(DT):
        nc.sync.dma_start(w_inT[i][:], w_inT_d[i * 128:(i + 1) * 128, :])
    cw = [wpool.tile([128, K], F32, tag=f"cw{i}", name=f"cw{i}") for i in range(ET)]
    cb = [wpool.tile([128, 1], F32, tag=f"cb{i}", name=f"cb{i}") for i in range(ET)]
    dtb = [wpool.tile([128, 1], F32, tag=f"dtb{i}", name=f"dtb{i}") for i in range(ET)]
    dp = [wpool.tile([128, 1], F32, tag=f"dp{i}", name=f"dp{i}") for i in range(ET)]
    aneg = [wpool.tile([128, N], F32, tag=f"aneg{i}", name=f"aneg{i}") for i in range(ET)]
    w_xT = [wpool.tile([128, R + 2 * N], FP16, tag=f"w_xT{i}", name=f"w_xT{i}") for i in range(ET)]
    w_outT = [wpool.tile([128, D], FP16, tag=f"w_outT{i}", name=f"w_outT{i}") for i in range(ET)]
    for i in range(ET):
        sl = slice(i * 128, (i + 1) * 128)
        nc.sync.dma_start(cw[i][:], cw_d[sl, :])
        nc.sync.dma_start(cb[i][:], cb_d[sl, :])
        nc.sync.dma_start(dtb[i][:], dtb_d[sl, :])
        nc.sync.dma_start(dp[i][:], dp_d[sl, :])
        nc.sync.dma_start(aneg[i][:], aneg_d[sl, :])
        nc.sync.dma_start(w_xT[i][:], w_xT_d[sl, :])
        nc.sync.dma_start(w_outT[i][:], w_outT_d[sl, :])
    w_dtT = wpool.tile([R, E], FP16, tag="w_dtT", name="w_dtT")
    nc.sync.dma_start(w_dtT[:], w_dtT_d[:])

    ident = wpool.tile([128, 128], F32, tag="ident", name="ident")
    masks.make_identity(nc, ident[:])
    identb = wpool.tile([128, 128], FP16, tag="identb", name="identb")
    nc.vector.tensor_copy(identb[:], ident[:])
    # flag-scaled identity: adds resid into the out_proj psum on self cores only
    flagident = wpool.tile([128, 128], FP16, tag="flagident", name="flagident")
    nc.vector.tensor_scalar_mul(flagident[:], ident[:], flagv[:])
    diag_dp = [wpool.tile([128, 128], FP16, tag=f"ddp{e}", name=f"ddp{e}") for e in range(ET)]
    for e in range(ET):
        nc.vector.tensor_scalar_mul(diag_dp[e][:], ident[:], dp[e][:])
    diag_cw = [[stage2.tile([128, 128], FP16, tag=f"dcw{e}_{k}", name=f"dcw{e}_{k}") for k in range(K)]
               for e in range(ET)]
    for e in range(ET):
        for k in range(K):
            nc.vector.tensor_scalar_mul(diag_cw[e][k][:], ident[:], cw[e][:, k:k + 1])

    # ---------------- stage1: input load + fp16 cast ----------------
    xin = [stage1.tile([128, L], F32, tag=f"xinf{i}", name=f"xinf{i}") for i in range(DT)]
    for i in range(DT):
        nc.sync.dma_start(xin[i][:], xin_d[i * 128:(i + 1) * 128, :])
    ffw = [wff.tile([128, K * D], FP16, tag=f"ffw{i}", name=f"ffw{i}") for i in range(DT)]
    for i in range(DT):
        nc.sync.dma_start(
            ffw[i][:].rearrange("p (k d) -> p k d", k=K),
            ffw_d[:, i * 128:(i + 1) * 128, :].rearrange("k p d -> p k d"),
        )
    # padded fp16 input (3 zeros each side for the K=7 same-padding conv)
    xin16 = [wff.tile([128, L + 6], FP16, tag=f"xin16_{i}", name=f"xin16_{i}") for i in range(DT)]
    for i in range(DT):
        nc.gpsimd.memset(xin16[i][:, 0:3], 0.0)
        nc.gpsimd.memset(xin16[i][:, L + 3:], 0.0)
        nc.vector.tensor_copy(xin16[i][:, 3:L + 3], xin[i][:])
    stage1.release()

    # ---------------- conv_ff -> convout (fp16) + resid (fp16) ----------------
    convout = [wff.tile([128, L], FP16, tag=f"convout{i}", name=f"convout{i}") for i in range(DT)]
    resid = [big.tile([128, L], FP16, tag=f"resid{i}", name=f"resid{i}") for i in range(DT)]
    for do in range(DT):
        for t in range(TC):
            ps = mmwp.tile([128, 512], F32, tag="mmw", name="psmmw")
            nmm = K * DT
            i = 0
            for k in range(K):
                for di in range(DT):
                    nc.tensor.matmul(
                        ps[:],
                        ffw[di][:, k * D + do * 128: k * D + (do + 1) * 128],
                        xin16[di][:, t * 512 + k: t * 512 + k + 512],
                        start=(i == 0), stop=(i == nmm - 1),
                    )
                    i += 1
            nc.scalar.activation(convout[do][:, t * 512:(t + 1) * 512], ps[:], AF.Relu, bias=ffb[do][:])
            nc.vector.tensor_tensor(
                resid[do][:, t * 512:(t + 1) * 512],
                convout[do][:, t * 512:(t + 1) * 512],
                xin16[do][:, 3 + t * 512: 3 + (t + 1) * 512], ADD,
            )

    # ---------------- instance norm -> mamba_in (fp16) ----------------
    # norm input: self cores = convout, cross cores = xin  (flagv selects)
    mamba_in = [stage3.tile([128, L], FP16, tag=f"mambain{i}", name=f"mambain{i}") for i in range(DT)]
    for i in range(DT):
        nin = chunk.tile([128, L], FP16, tag="nin2048", name="nin")
        nc.vector.tensor_scalar_mul(nin[:], xin16[i][:, 3:L + 3], flag2[:])
        nc.vector.scalar_tensor_tensor(nin[:], convout[i][:], flagv[:], nin[:], MULT, ADD)
        ssum = wpool.tile([128, 1], F32, tag=f"ssum{i}", name=f"ssum{i}")
        ssq = wpool.tile([128, 1], F32, tag=f"ssq{i}", name=f"ssq{i}")
        trash = chunk.tile([128, L], FP16, tag="nin2048", name="trash")
        nc.vector.tensor_reduce(ssum[:], nin[:], mybir.AxisListType.X, ADD)
        nc.scalar.activation(trash[:], nin[:], AF.Square, accum_out=ssq[:])
        mean = wpool.tile([128, 1], F32, tag=f"mean{i}", name=f"mean{i}")
        var = wpool.tile([128, 1], F32, tag=f"var{i}", name=f"var{i}")
        nc.vector.tensor_scalar_mul(mean[:], ssum[:], 1.0 / L)
        nc.vector.tensor_scalar_mul(var[:], ssq[:], 1.0 / L)
        msq = wpool.tile([128, 1], F32, tag=f"msq{i}", name=f"msq{i}")
        nc.vector.tensor_tensor(msq[:], mean[:], mean[:], MULT)
        nc.vector.scalar_tensor_tensor(var[:], msq[:], -1.0, var[:], MULT, ADD)
        nc.vector.tensor_scalar_add(var[:], var[:], EPS)
        inv = wpool.tile([128, 1], F32, tag=f"inv{i}", name=f"inv{i}")
        nc.vector.reciprocal(inv[:], var[:])
        nc.scalar.sqrt(inv[:], inv[:])
        muinv = wpool.tile([128, 1], F32, tag=f"muinv{i}", name=f"muinv{i}")
        nc.vector.tensor_tensor(muinv[:], mean[:], inv[:], MULT)
        nc.vector.tensor_scalar(mamba_in[i][:], nin[:], inv[:], muinv[:], MULT, SUB)
    wff.release()

    # ---------------- in_proj (xp half) ----------------
    xp = [stage2.tile([128, L + 6], FP16, tag=f"xp{e}", name=f"xp{e}") for e in range(ET)]
    for e in range(ET):
        nc.gpsimd.memset(xp[e][:, 0:6], 0.0)
        for t in range(TC):
            ps = mmwp.tile([128, 512], F32, tag="mmw", name="psmmw")
            for di in range(DT):
                nc.tensor.matmul(
                    ps[:], w_inT[di][:, e * 128:(e + 1) * 128],
                    mamba_in[di][:, t * 512:(t + 1) * 512],
                    start=(di == 0), stop=(di == DT - 1),
                )
            nc.scalar.copy(xp[e][:, 6 + t * 512: 6 + (t + 1) * 512], ps[:])

    # ---------------- gate half: zs = silu(in_proj_z) ----------------
    zs = [big.tile([128, L], FP16, tag=f"zs{e}", name=f"zs{e}") for e in range(ET)]
    for e in range(ET):
        for t in range(TC):
            ps = psmall.tile([128, 512], F32, tag="mm", name="psmm")
            for di in range(DT):
                nc.tensor.matmul(
                    ps[:], w_inT[di][:, E + e * 128: E + (e + 1) * 128],
                    mamba_in[di][:, t * 512:(t + 1) * 512],
                    start=(di == 0), stop=(di == DT - 1),
                )
            nc.scalar.activation(zs[e][:, t * 512:(t + 1) * 512], ps[:], AF.Silu)
    stage3.release()

    # ---------------- depthwise causal conv + silu -> u ----------------
    u = [big.tile([128, L], FP16, tag=f"u{e}", name=f"u{e}") for e in range(ET)]
    for e in range(ET):
        for t in range(TC):
            ps = mmwp.tile([128, 512], F32, tag="mmw", name="psmmw")
            for k in range(K):
                nc.tensor.matmul(
                    ps[:], diag_cw[e][k][:],
                    xp[e][:, t * 512 + k: t * 512 + k + 512],
                    start=(k == 0), stop=(k == K - 1),
                )
            nc.scalar.activation(u[e][:, t * 512:(t + 1) * 512], ps[:], AF.Silu, bias=cb[e][:])
    stage2.release()

    # ---------------- x_proj -> xdblR (fp16) + B/C rows to DRAM ----------------
    xdblR = big.tile([R, L], FP16, tag="xdblR", name="xdblR")
    bcsrc = big.tile([2 * N, L], FP16, tag="bcsrc", name="bcsrc")
    for t in range(TC):
        ps = mmwp.tile([R + 2 * N, 512], F32, tag="mmw", name="psmmx")
        for e in range(ET):
            nc.tensor.matmul(
                ps[:], w_xT[e][:], u[e][:, t * 512:(t + 1) * 512],
                start=(e == 0), stop=(e == ET - 1),
            )
        nc.scalar.copy(bcsrc[:, t * 512:(t + 1) * 512], ps[0:2 * N, :])
        nc.scalar.copy(xdblR[:, t * 512:(t + 1) * 512], ps[2 * N:2 * N + R, :])
    bc_dram = dramp.tile([2 * N, L], FP16, tag="bcdram", name="bcdram")
    nc.sync.dma_start(bc_dram[:], bcsrc[:])

    # ---------------- dt_proj -> dt = -softplus(...) via ln(sigmoid(-x)) ----------------
    # (no Softplus table on gen3; sigmoid/ln batched per e-tile to limit table loads)
    dt = [big.tile([128, L], FP16, tag=f"dt{e}", name=f"dt{e}") for e in range(ET)]
    for e in range(ET):
        lntmp = chunk.tile([128, L], F32, tag="lntmp", name=f"lntmp{e}")
        for t in range(TC):
            ps = psmall.tile([128, 512], F32, tag="mm", name="psmm")
            nc.tensor.matmul(
                ps[:], w_dtT[:, e * 128:(e + 1) * 128],
                xdblR[:, t * 512:(t + 1) * 512], start=True, stop=True,
            )
            nc.scalar.activation(lntmp[:, t * 512:(t + 1) * 512], ps[:], AF.Sigmoid,
                                 bias=dtb[e][:], scale=-1.0)
        for t in range(TC):
            nc.scalar.activation(dt[e][:, t * 512:(t + 1) * 512],
                                 lntmp[:, t * 512:(t + 1) * 512], AF.Ln)

    # ---------------- w = dt * u ----------------
    w = [big.tile([128, L], FP16, tag=f"w{e}", name=f"w{e}") for e in range(ET)]
    for e in range(ET):
        nc.vector.tensor_tensor(w[e][:], dt[e][:], u[e][:], MULT)

    # ---------------- selective scan ----------------
    mmwp.release()
    pbig = pool("psum_y", 1, space="PSUM")       # [128,2048] y accumulator
    scanp = pool("scan", 3)
    bcp = pool("bcast", 3)
    yg = []
    for e in range(ET):
        py = pbig.tile([128, L], F32, tag="py", name=f"py{e}")
        for t in range(TC):
            nc.tensor.matmul(
                py[:, t * 512:(t + 1) * 512], diag_dp[e][:],
                u[e][:, t * 512:(t + 1) * 512],
                start=True, stop=False,
            )
        for n in range(N):
            b_bc = bcp.tile([128, L], FP16, tag="bbc", name="bbc")
            c_bc = bcp.tile([128, L], FP16, tag="cbc", name="cbc")
            nc.sync.dma_start(b_bc[:], bc_dram[n, :].partition_broadcast(128))
            nc.sync.dma_start(c_bc[:], bc_dram[N + n, :].partition_broadcast(128))
            dA = scanp.tile([128, L], FP16, tag="dA", name="dA")
            nc.scalar.activation(dA[:], dt[e][:], AF.Exp, scale=aneg[e][:, n:n + 1])
            z = scanp.tile([128, L], FP16, tag="z", name="z")
            nc.vector.tensor_tensor(z[:], w[e][:], b_bc[:], MULT)
            h = scanp.tile([128, L], FP16, tag="h", name="h")
            nc.vector.tensor_tensor_scan(h[:], dA[:], z[:], 0.0, MULT, ADD)
            yp = scanp.tile([128, L], FP16, tag="yp", name="yp")
            nc.vector.tensor_tensor(yp[:], h[:], c_bc[:], MULT)
            for t in range(TC):
                nc.tensor.matmul(
                    py[:, t * 512:(t + 1) * 512], identb[:],
                    yp[:, t * 512:(t + 1) * 512],
                    start=False, stop=(n == N - 1),
                )
        # ---- gating: yg = py * zs (silu of gate, precomputed) ----
        yge = big.tile([128, L], FP16, tag=f"u{e}", name=f"yg{e}")
        yg.append(yge)
        for t in range(TC):
            nc.vector.tensor_tensor(
                yge[:, t * 512:(t + 1) * 512],
                py[:, t * 512:(t + 1) * 512],
                zs[e][:, t * 512:(t + 1) * 512], MULT,
            )

    # ---------------- out_proj + resid-add (via matmul) + mask + store ----------------
    for do in range(DT):
        for t in range(TC):
            ps = psmall.tile([128, 512], F32, tag="mm", name="psmm")
            for e in range(ET):
                nc.tensor.matmul(
                    ps[:], w_outT[e][:, do * 128:(do + 1) * 128],
                    yg[e][:, t * 512:(t + 1) * 512],
                    start=(e == 0), stop=False,
                )
            nc.tensor.matmul(
                ps[:], flagident[:],
                resid[do][:, t * 512:(t + 1) * 512],
                start=False, stop=True,
            )
            ofin = chunk.tile([128, 512], F32, tag="ofin", name="ofin")
            nc.vector.tensor_tensor(ofin[:], ps[:], maskb[:, t * 512:(t + 1) * 512], MULT)
            nc.sync.dma_start(out_d[do * 128:(do + 1) * 128, t * 512:(t + 1) * 512], ofin[:])

    for p in reversed(_pools):
        if not p._released:
            p.release()


_NC_CACHE = {}


def _get_nc():
    if "nc" not in _NC_CACHE:
        _NC_CACHE["nc"] = build_nc()
    return _NC_CACHE["nc"]


def _core_inputs(blk, b, inputs):
    pfx = "s_" if blk == 0 else "c_"
    xin = inputs["x"][b] if blk == 0 else inputs["encoder_states"][b]
    f = 1.0 if blk == 0 else 0.0
    g = lambda k: np.asarray(inputs[pfx + k])
    aneg = np.exp(g("A_log"))  # = -A; dt tile holds -softplus so dA = exp(aneg*dt)
    return {
        "xin": np.ascontiguousarray(xin, np.float32),
        "maskb": np.ascontiguousarray(
            np.broadcast_to(inputs["padding_mask"][b][None, :], (128, L))).astype(np.float16),
        "flagv": np.full((128, 1), f, np.float32),
        "flag2": np.full((128, 1), 1.0 - f, np.float32),
        "ffw": np.ascontiguousarray(np.asarray(inputs["ff_w"]).transpose(2, 1, 0)).astype(np.float16),
        "ffb": np.asarray(inputs["ff_b"]).reshape(D, 1).astype(np.float32),
        "w_inT": np.ascontiguousarray(g("in_proj_w").T).astype(np.float16),
        "cw": np.ascontiguousarray(g("conv_w").reshape(E, K), np.float32),
        "cb": g("conv_b").reshape(E, 1).astype(np.float32),
        "w_xT": np.ascontiguousarray(
            g("x_proj_w").T[:, list(range(R, R + 2 * N)) + list(range(R))]
        ).astype(np.float16),
        "w_dtT": np.ascontiguousarray(g("dt_proj_w").T).astype(np.float16),
        "dtb": (-g("dt_proj_b")).reshape(E, 1).astype(np.float32),
        "aneg": np.ascontiguousarray(aneg, np.float32),
        "dp": (-g("D")).reshape(E, 1).astype(np.float32),
        "w_outT": np.ascontiguousarray(-g("out_proj_w").T).astype(np.float16),
    }


def kernel(**inputs):
    nc = _get_nc()
    in_maps = []
    for b in range(B):
        in_maps.append(_core_inputs(0, b, inputs))  # core 2b: self block
        in_maps.append(_core_inputs(1, b, inputs))  # core 2b+1: cross block
    res = run_bass_kernel_spmd(nc, in_maps, core_ids=list(range(8)))
    out = np.empty((B, D, L), np.float32)
    for b in range(B):
        out[b] = res.results[2 * b]["out"] + res.results[2 * b + 1]["out"]
    return out


# revision 3
# speedup vs baseline: 1.0128x; 1.0122x over previous
"""Trainium2 Bass kernel for nn_AttModule_mamba_cross (B=4,D=256,L=2048,E=512,N=16,K=7,R=16).

Sharding: 8 cores = 2 mamba blocks x 4 batches, one (block, batch) unit per core.
All cores run one SPMD program; a per-core flag selects whether the conv_ff(x)
path is blended in (self-block cores) or the raw input is used (cross-block
cores). Host sums each core pair's partial outputs.

V-engine-optimized rewrite: the DVE is the bottleneck (64 scans + z/yp muls),
so everything else is pushed to Scalar (Silu/Softplus/Exp activations) and
Tensor (identity-matmul accumulation), fp16 throughout, and the gate/silu
work is hoisted out of the scan loop so Scalar runs Exp-only there.
padding_mask is all-ones per the problem spec (fill: ones); it is applied
only at the final output store.
"""
import numpy as np

import concourse.bass as bass
import concourse.bacc as bacc
import concourse.mybir as mybir
import concourse.tile as tile
from concourse import masks
from concourse.bass_utils import run_bass_kernel_spmd

B, D, L = 4, 256, 2048
E, N, K = 512, 16, 7
R = 16
EPS = 1e-5

F32 = mybir.dt.float32
FP16 = mybir.dt.float16
MULT = mybir.AluOpType.mult
ADD = mybir.AluOpType.add
SUB = mybir.AluOpType.subtract
AF = mybir.ActivationFunctionType

DT = D // 128   # 2 d-tiles
ET = E // 128   # 4 e-tiles
TC = L // 512   # 4 t-chunks of 512


def build_nc():
    nc = bacc.Bacc("TRN2", target_bir_lowering=False, debug=False, num_devices=8)

    # ---- DRAM I/O ----
    xin_d = nc.dram_tensor("xin", [D, L], F32, kind="ExternalInput")
    maskb_d = nc.dram_tensor("maskb", [128, L], FP16, kind="ExternalInput")
    flagv_d = nc.dram_tensor("flagv", [128, 1], F32, kind="ExternalInput")
    flag2_d = nc.dram_tensor("flag2", [128, 1], F32, kind="ExternalInput")
    ffw_d = nc.dram_tensor("ffw", [K, D, D], FP16, kind="ExternalInput")
    ffb_d = nc.dram_tensor("ffb", [D, 1], F32, kind="ExternalInput")
    w_inT_d = nc.dram_tensor("w_inT", [D, 2 * E], FP16, kind="ExternalInput")
    cw_d = nc.dram_tensor("cw", [E, K], F32, kind="ExternalInput")
    cb_d = nc.dram_tensor("cb", [E, 1], F32, kind="ExternalInput")
    w_xT_d = nc.dram_tensor("w_xT", [E, R + 2 * N], FP16, kind="ExternalInput")
    w_dtT_d = nc.dram_tensor("w_dtT", [R, E], FP16, kind="ExternalInput")
    dtb_d = nc.dram_tensor("dtb", [E, 1], F32, kind="ExternalInput")
    aneg_d = nc.dram_tensor("aneg", [E, N], F32, kind="ExternalInput")
    dp_d = nc.dram_tensor("dp", [E, 1], F32, kind="ExternalInput")
    w_outT_d = nc.dram_tensor("w_outT", [E, D], FP16, kind="ExternalInput")
    out_d = nc.dram_tensor("out", [D, L], F32, kind="ExternalOutput")

    with tile.TileContext(nc) as tc:
        _emit(nc, tc, locals())
    nc.compile()
    return nc


def _emit(nc, tc, d):
    xin_d, maskb_d, flagv_d, flag2_d = d["xin_d"], d["maskb_d"], d["flagv_d"], d["flag2_d"]
    ffw_d, ffb_d, w_inT_d = d["ffw_d"], d["ffb_d"], d["w_inT_d"]
    cw_d, cb_d, w_xT_d, w_dtT_d = d["cw_d"], d["cb_d"], d["w_xT_d"], d["w_dtT_d"]
    dtb_d, aneg_d, dp_d, w_outT_d, out_d = d["dtb_d"], d["aneg_d"], d["dp_d"], d["w_outT_d"], d["out_d"]

    _pools = []

    def pool(name, bufs, space="SBUF"):
        p = tc.alloc_tile_pool(name=name, bufs=bufs, space=space)
        _pools.append(p)
        return p

    wpool = pool("weights", 1)          # persistent small weights/constants
    big = pool("big", 1)                # persistent big activations
    chunk = pool("chunk512", 2)         # transient [128,512] tiles
    psmall = pool("psum_mm", 2, space="PSUM")    # [128,512] matmul tiles
    mmwp = pool("psum_mmw", 2, space="PSUM")     # [128,512] prelude matmul tiles
    dramp = pool("dram", 1, space="DRAM")
    # stage pools, created in reverse order of release (stack allocator is LIFO)
    stage2 = pool("stage2", 1)          # xp halo + diag_cw (released after dwconv)
    stage3 = pool("stage3", 1)          # mamba_in (released after gate)
    wff = pool("wff", 1)                # conv_ff weights + xin16 + convout (released after norm)
    stage1 = pool("stage1", 1)          # xin f32 (released after cast)

    # ---------------- persistent weights ----------------
    maskb = wpool.tile([128, L], FP16, tag="maskb", name="maskb")
    nc.sync.dma_start(maskb[:], maskb_d[:])
    flagv = wpool.tile([128, 1], F32, tag="flagv", name="flagv")
    flag2 = wpool.tile([128, 1], F32, tag="flag2", name="flag2")
    nc.sync.dma_start(flagv[:], flagv_d[:])
    nc.sync.dma_start(flag2[:], flag2_d[:])
    ffb = [wpool.tile([128, 1], F32, tag=f"ffb{i}", name=f"ffb{i}") for i in range(DT)]
    for i in range(DT):
        nc.sync.dma_start(ffb[i][:], ffb_d[i * 128:(i + 1) * 128, :])
    w_inT = [wpool.tile([128, 2 * E], FP16, tag=f"w_inT{i}", name=f"w_inT{i}") for i in range(DT)]
    for i in range(DT):
        nc.sync.dma_start(w_inT[i][:], w_inT_d[i * 128:(i + 1) * 128, :])
    cw = [wpool.tile([128, K], F32, tag=f"cw{i}", name=f"cw{i}") for i in range(ET)]
    cb = [wpool.tile([128, 1], F32, tag=f"cb{i}", name=f"cb{i}") for i in range(ET)]
    dtb = [wpool.tile([128, 1], F32, tag=f"dtb{i}", name=f"dtb{i}") for i in range(ET)]
    dp = [wpool.tile([128, 1], F32, tag=f"dp{i}", name=f"dp{i}") for i in range(ET)]
    aneg = [wpool.tile([128, N], F32, tag=f"aneg{i}", name=f"aneg{i}") for i in range(ET)]
    w_xT = [wpool.tile([128, R + 2 * N], FP16, tag=f"w_xT{i}", name=f"w_xT{i}") for i in range(ET)]
    w_outT = [wpool.tile([128, D], FP16, tag=f"w_outT{i}", name=f"w_outT{i}") for i in range(ET)]
    for i in range(ET):
        sl = slice(i * 128, (i + 1) * 128)
        nc.sync.dma_start(cw[i][:], cw_d[sl, :])
        nc.sync.dma_start(cb[i][:], cb_d[sl, :])
        nc.sync.dma_start(dtb[i][:], dtb_d[sl, :])
        nc.sync.dma_start(dp[i][:], dp_d[sl, :])
        nc.sync.dma_start(aneg[i][:], aneg_d[sl, :])
        nc.sync.dma_start(w_xT[i][:], w_xT_d[sl, :])
        nc.sync.dma_start(w_outT[i][:], w_outT_d[sl, :])
    w_dtT = wpool.tile([R, E], FP16, tag="w_dtT", name="w_dtT")
    nc.sync.dma_start(w_dtT[:], w_dtT_d[:])

    ident = wpool.tile([128, 128], F32, tag="ident", name="ident")
    masks.make_identity(nc, ident[:])
    identb = wpool.tile([128, 128], FP16, tag="identb", name="identb")
    nc.vector.tensor_copy(identb[:], ident[:])
    # flag-scaled identity: adds resid into the out_proj psum on self cores only
    flagident = wpool.tile([128, 128], FP16, tag="flagident", name="flagident")
    nc.vector.tensor_scalar_mul(flagident[:], ident[:], flagv[:])
    diag_dp = [wpool.tile([128, 128], FP16, tag=f"ddp{e}", name=f"ddp{e}") for e in range(ET)]
    for e in range(ET):
        nc.vector.tensor_scalar_mul(diag_dp[e][:], ident[:], dp[e][:])
    diag_cw = [[stage2.tile([128, 128], FP16, tag=f"dcw{e}_{k}", name=f"dcw{e}_{k}") for k in range(K)]
               for e in range(ET)]
    for e in range(ET):
        for k in range(K):
            nc.vector.tensor_scalar_mul(diag_cw[e][k][:], ident[:], cw[e][:, k:k + 1])

    # ---------------- stage1: input load + fp16 cast ----------------
    xin = [stage1.tile([128, L], F32, tag=f"xinf{i}", name=f"xinf{i}") for i in range(DT)]
    for i in range(DT):
        nc.sync.dma_start(xin[i][:], xin_d[i * 128:(i + 1) * 128, :])
    ffw = [wff.tile([128, K * D], FP16, tag=f"ffw{i}", name=f"ffw{i}") for i in range(DT)]
    for i in range(DT):
        nc.sync.dma_start(
            ffw[i][:].rearrange("p (k d) -> p k d", k=K),
            ffw_d[:, i * 128:(i + 1) * 128, :].rearrange("k p d -> p k d"),
        )
    # padded fp16 input (3 zeros each side for the K=7 same-padding conv)
    xin16 = [wff.tile([128, L + 6], FP16, tag=f"xin16_{i}", name=f"xin16_{i}") for i in range(DT)]
    for i in range(DT):
        nc.gpsimd.memset(xin16[i][:, 0:3], 0.0)
        nc.gpsimd.memset(xin16[i][:, L + 3:], 0.0)
        nc.vector.tensor_copy(xin16[i][:, 3:L + 3], xin[i][:])
    stage1.release()

    # ---------------- conv_ff -> convout (fp16) + resid (fp16) ----------------
    convout = [wff.tile([128, L], FP16, tag=f"convout{i}", name=f"convout{i}") for i in range(DT)]
    resid = [big.tile([128, L], FP16, tag=f"resid{i}", name=f"resid{i}") for i in range(DT)]
    for do in range(DT):
        for t in range(TC):
            ps = mmwp.tile([128, 512], F32, tag="mmw", name="psmmw")
            nmm = K * DT
            i = 0
            for k in range(K):
                for di in range(DT):
                    nc.tensor.matmul(
                        ps[:],
                        ffw[di][:, k * D + do * 128: k * D + (do + 1) * 128],
                        xin16[di][:, t * 512 + k: t * 512 + k + 512],
                        start=(i == 0), stop=(i == nmm - 1),
                    )
                    i += 1
            nc.scalar.activation(convout[do][:, t * 512:(t + 1) * 512], ps[:], AF.Relu, bias=ffb[do][:])
            nc.vector.tensor_tensor(
                resid[do][:, t * 512:(t + 1) * 512],
                convout[do][:, t * 512:(t + 1) * 512],
                xin16[do][:, 3 + t * 512: 3 + (t + 1) * 512], ADD,
            )

    # ---------------- instance norm -> mamba_in (fp16) ----------------
    # norm input: self cores = convout, cross cores = xin  (flagv selects)
    mamba_in = [stage3.tile([128, L], FP16, tag=f"mambain{i}", name=f"mambain{i}") for i in range(DT)]
    for i in range(DT):
        nin = chunk.tile([128, L], FP16, tag="nin2048", name="nin")
        nc.vector.tensor_scalar_mul(nin[:], xin16[i][:, 3:L + 3], flag2[:])
        nc.vector.scalar_tensor_tensor(nin[:], convout[i][:], flagv[:], nin[:], MULT, ADD)
        ssum = wpool.tile([128, 1], F32, tag=f"ssum{i}", name=f"ssum{i}")
        ssq = wpool.tile([128, 1], F32, tag=f"ssq{i}", name=f"ssq{i}")
        trash = chunk.tile([128, L], FP16, tag="nin2048", name="trash")
        nc.vector.tensor_reduce(ssum[:], nin[:], mybir.AxisListType.X, ADD)
        nc.scalar.activation(trash[:], nin[:], AF.Square, accum_out=ssq[:])
        mean = wpool.tile([128, 1], F32, tag=f"mean{i}", name=f"mean{i}")
        var = wpool.tile([128, 1], F32, tag=f"var{i}", name=f"var{i}")
        nc.vector.tensor_scalar_mul(mean[:], ssum[:], 1.0 / L)
        nc.vector.tensor_scalar_mul(var[:], ssq[:], 1.0 / L)
        msq = wpool.tile([128, 1], F32, tag=f"msq{i}", name=f"msq{i}")
        nc.vector.tensor_tensor(msq[:], mean[:], mean[:], MULT)
        nc.vector.scalar_tensor_tensor(var[:], msq[:], -1.0, var[:], MULT, ADD)
        nc.vector.tensor_scalar_add(var[:], var[:], EPS)
        inv = wpool.tile([128, 1], F32, tag=f"inv{i}", name=f"inv{i}")
        nc.vector.reciprocal(inv[:], var[:])
        nc.scalar.sqrt(inv[:], inv[:])
        muinv = wpool.tile([128, 1], F32, tag=f"muinv{i}", name=f"muinv{i}")
        nc.vector.tensor_tensor(muinv[:], mean[:], inv[:], MULT)
        nc.vector.tensor_scalar(mamba_in[i][:], nin[:], inv[:], muinv[:], MULT, SUB)
    wff.release()

    # ---------------- in_proj (xp half) ----------------
    xp = [stage2.tile([128, L + 6], FP16, tag=f"xp{e}", name=f"xp{e}") for e in range(ET)]
    for e in range(ET):
        nc.gpsimd.memset(xp[e][:, 0:6], 0.0)
        for t in range(TC):
            ps = mmwp.tile([128, 512], F32, tag="mmw", name="psmmw")
            for di in range(DT):
                nc.tensor.matmul(
                    ps[:], w_inT[di][:, e * 128:(e + 1) * 128],
                    mamba_in[di][:, t * 512:(t + 1) * 512],
                    start=(di == 0), stop=(di == DT - 1),
                )
            nc.scalar.copy(xp[e][:, 6 + t * 512: 6 + (t + 1) * 512], ps[:])

    # ---------------- gate half: zs = silu(in_proj_z) ----------------
    zs = [big.tile([128, L], FP16, tag=f"zs{e}", name=f"zs{e}") for e in range(ET)]
    for e in range(ET):
        for t in range(TC):
            ps = psmall.tile([128, 512], F32, tag="mm", name="psmm")
            for di in range(DT):
                nc.tensor.matmul(
                    ps[:], w_inT[di][:, E + e * 128: E + (e + 1) * 128],
                    mamba_in[di][:, t * 512:(t + 1) * 512],
                    start=(di == 0), stop=(di == DT - 1),
                )
            nc.scalar.activation(zs[e][:, t * 512:(t + 1) * 512], ps[:], AF.Silu)
    stage3.release()

    # ---------------- depthwise causal conv + silu -> u ----------------
    u = [big.tile([128, L], FP16, tag=f"u{e}", name=f"u{e}") for e in range(ET)]
    for e in range(ET):
        for t in range(TC):
            ps = mmwp.tile([128, 512], F32, tag="mmw", name="psmmw")
            for k in range(K):
                nc.tensor.matmul(
                    ps[:], diag_cw[e][k][:],
                    xp[e][:, t * 512 + k: t * 512 + k + 512],
                    start=(k == 0), stop=(k == K - 1),
                )
            nc.scalar.activation(u[e][:, t * 512:(t + 1) * 512], ps[:], AF.Silu, bias=cb[e][:])
    stage2.release()

    # ---------------- x_proj -> xdblR (fp16) + B/C rows to DRAM ----------------
    xdblR = big.tile([R, L], FP16, tag="xdblR", name="xdblR")
    bcsrc = big.tile([2 * N, L], FP16, tag="bcsrc", name="bcsrc")
    for t in range(TC):
        ps = mmwp.tile([R + 2 * N, 512], F32, tag="mmw", name="psmmx")
        for e in range(ET):
            nc.tensor.matmul(
                ps[:], w_xT[e][:], u[e][:, t * 512:(t + 1) * 512],
                start=(e == 0), stop=(e == ET - 1),
            )
        nc.scalar.copy(bcsrc[:, t * 512:(t + 1) * 512], ps[0:2 * N, :])
        nc.scalar.copy(xdblR[:, t * 512:(t + 1) * 512], ps[2 * N:2 * N + R, :])
    bc_dram = dramp.tile([2 * N, L], FP16, tag="bcdram", name="bcdram")
    nc.sync.dma_start(bc_dram[:], bcsrc[:])

    # ---------------- dt_proj -> dt = -softplus(...) via ln(sigmoid(-x)) ----------------
    # (no Softplus table on gen3; sigmoid/ln batched per e-tile to limit table loads)
    dt = [big.tile([128, L], FP16, tag=f"dt{e}", name=f"dt{e}") for e in range(ET)]
    for e in range(ET):
        lntmp = chunk.tile([128, L], F32, tag="lntmp", name=f"lntmp{e}")
        for t in range(TC):
            ps = psmall.tile([128, 512], F32, tag="mm", name="psmm")
            nc.tensor.matmul(
                ps[:], w_dtT[:, e * 128:(e + 1) * 128],
                xdblR[:, t * 512:(t + 1) * 512], start=True, stop=True,
            )
            nc.scalar.activation(lntmp[:, t * 512:(t + 1) * 512], ps[:], AF.Sigmoid,
                                 bias=dtb[e][:], scale=-1.0)
        for t in range(TC):
            nc.scalar.activation(dt[e][:, t * 512:(t + 1) * 512],
                                 lntmp[:, t * 512:(t + 1) * 512], AF.Ln)

    # ---------------- w = dt * u ----------------
    w = [big.tile([128, L], FP16, tag=f"w{e}", name=f"w{e}") for e in range(ET)]
    for e in range(ET):
        nc.vector.tensor_tensor(w[e][:], dt[e][:], u[e][:], MULT)

    # ---------------- selective scan ----------------
    mmwp.release()
    pbig = pool("psum_y", 1, space="PSUM")       # [128,2048] y accumulator
    scanp = pool("scan", 3)
    bcp = pool("bcast", 4)
    yg = []
    for e in range(ET):
        py = pbig.tile([128, L], F32, tag="py", name=f"py{e}")
        for t in range(TC):
            nc.tensor.matmul(
                py[:, t * 512:(t + 1) * 512], diag_dp[e][:],
                u[e][:, t * 512:(t + 1) * 512],
                start=True, stop=False,
            )
        for n in range(N):
            b_bc = bcp.tile([128, L], FP16, tag="bbc", name="bbc")
            c_bc = bcp.tile([128, L], FP16, tag="cbc", name="cbc")
            nc.sync.dma_start(b_bc[:], bc_dram[n, :].partition_broadcast(128))
            nc.sync.dma_start(c_bc[:], bc_dram[N + n, :].partition_broadcast(128))
            dA = scanp.tile([128, L], FP16, tag="dA", name="dA")
            nc.scalar.activation(dA[:], dt[e][:], AF.Exp, scale=aneg[e][:, n:n + 1])
            z = scanp.tile([128, L], FP16, tag="z", name="z")
            nc.vector.tensor_tensor(z[:], w[e][:], b_bc[:], MULT)
            h = scanp.tile([128, L], FP16, tag="h", name="h")
            nc.vector.tensor_tensor_scan(h[:], dA[:], z[:], 0.0, MULT, ADD)
            yp = scanp.tile([128, L], FP16, tag="yp", name="yp")
            nc.vector.tensor_tensor(yp[:], h[:], c_bc[:], MULT)
            for t in range(TC):
                nc.tensor.matmul(
                    py[:, t * 512:(t + 1) * 512], identb[:],
                    yp[:, t * 512:(t + 1) * 512],
                    start=False, stop=(n == N - 1),
                )
        # ---- gating: yg = py * zs (silu of gate, precomputed) ----
        yge = big.tile([128, L], FP16, tag=f"u{e}", name=f"yg{e}")
        yg.append(yge)
        for t in range(TC):
            nc.vector.tensor_tensor(
                yge[:, t * 512:(t + 1) * 512],
                py[:, t * 512:(t + 1) * 512],
                zs[e][:, t * 512:(t + 1) * 512], MULT,
            )

    # ---------------- out_proj + resid-add (via matmul) + mask + store ----------------
    for do in range(DT):
        for t in range(TC):
            ps = psmall.tile([128, 512], F32, tag="mm", name="psmm")
            for e in range(ET):
                nc.tensor.matmul(
                    ps[:], w_outT[e][:, do * 128:(do + 1) * 128],
                    yg[e][:, t * 512:(t + 1) * 512],
                    start=(e == 0), stop=False,
                )
            nc.tensor.matmul(
                ps[:], flagident[:],
                resid[do][:, t * 512:(t + 1) * 512],
                start=False, stop=True,
            )
            ofin = chunk.tile([128, 512], F32, tag="ofin", name="ofin")
            nc.vector.tensor_tensor(ofin[:], ps[:], maskb[:, t * 512:(t + 1) * 512], MULT)
            nc.sync.dma_start(out_d[do * 128:(do + 1) * 128, t * 512:(t + 1) * 512], ofin[:])

    for p in reversed(_pools):
        if not p._released:
            p.release()


_NC_CACHE = {}


def _get_nc():
    if "nc" not in _NC_CACHE:
        _NC_CACHE["nc"] = build_nc()
    return _NC_CACHE["nc"]


def _core_inputs(blk, b, inputs):
    pfx = "s_" if blk == 0 else "c_"
    xin = inputs["x"][b] if blk == 0 else inputs["encoder_states"][b]
    f = 1.0 if blk == 0 else 0.0
    g = lambda k: np.asarray(inputs[pfx + k])
    aneg = np.exp(g("A_log"))  # = -A; dt tile holds -softplus so dA = exp(aneg*dt)
    return {
        "xin": np.ascontiguousarray(xin, np.float32),
        "maskb": np.ascontiguousarray(
            np.broadcast_to(inputs["padding_mask"][b][None, :], (128, L))).astype(np.float16),
        "flagv": np.full((128, 1), f, np.float32),
        "flag2": np.full((128, 1), 1.0 - f, np.float32),
        "ffw": np.ascontiguousarray(np.asarray(inputs["ff_w"]).transpose(2, 1, 0)).astype(np.float16),
        "ffb": np.asarray(inputs["ff_b"]).reshape(D, 1).astype(np.float32),
        "w_inT": np.ascontiguousarray(g("in_proj_w").T).astype(np.float16),
        "cw": np.ascontiguousarray(g("conv_w").reshape(E, K), np.float32),
        "cb": g("conv_b").reshape(E, 1).astype(np.float32),
        "w_xT": np.ascontiguousarray(
            g("x_proj_w").T[:, list(range(R, R + 2 * N)) + list(range(R))]
        ).astype(np.float16),
        "w_dtT": np.ascontiguousarray(g("dt_proj_w").T).astype(np.float16),
        "dtb": (-g("dt_proj_b")).reshape(E, 1).astype(np.float32),
        "aneg": np.ascontiguousarray(aneg, np.float32),
        "dp": (-g("D")).reshape(E, 1).astype(np.float32),
        "w_outT": np.ascontiguousarray(-g("out_proj_w").T).astype(np.float16),
    }


def kernel(**inputs):
    nc = _get_nc()
    in_maps = []
    for b in range(B):
        in_maps.append(_core_inputs(0, b, inputs))  # core 2b: self block
        in_maps.append(_core_inputs(1, b, inputs))  # core 2b+1: cross block
    res = run_bass_kernel_spmd(nc, in_maps, core_ids=list(range(8)))
    out = np.empty((B, D, L), np.float32)
    for b in range(B):
        out[b] = res.results[2 * b]["out"] + res.results[2 * b + 1]["out"]
    return out
